# revision 17
# baseline (speedup 1.0000x reference)
"""Trainium2 Bass kernel for nn_CWVAE (2-level clockwork VAE loss).

Sharding: data-parallel over batch B=64 across 8 cores (8 samples/core);
parameters replicated; per-sample partial losses gathered and reduced on host.

On-chip layout is feature-major ("F-major"): features on the 128 SBUF
partitions, columns indexed by c = t*NB + b (time-major, batch-inner).
All matmuls keep weights stationary (lhsT = W[in, out]) and stream
activation columns, so no on-chip transposes are needed anywhere.

The sequential GRU scans use a single ACT table set (natural_log_exp:
Exp/Ln/Relu/Identity/Copy): sigmoid and tanh are computed exactly as
1/(1+exp(-x)) and 1-2/(1+exp(2x)) with the fast-Newton DVE reciprocal
(~2 ULP), softplus as Ln(1+Exp(x)) — this avoids any per-step ACT
table switching (~2.7us each).
"""

import math
import numpy as np

import concourse.bass as bass
import concourse.bacc as bacc
import concourse.mybir as mybir
from concourse.bass import ds, ts
from concourse.tile import TileContext

F32 = mybir.dt.float32
BF16 = mybir.dt.bfloat16
I32 = mybir.dt.int32
AF = mybir.ActivationFunctionType
OP = mybir.AluOpType

LOG2PI = float(np.log(2.0 * np.pi))

# model dims
Z, H, E, D = 128, 256, 256, 256
B, T = 64, 1024
NCORES = 8
NB = B // NCORES  # 8 samples per core


class Cfg:
    def __init__(self, T0=1024, HC=32, mm_bf16=True):
        self.T0 = T0          # level-0 sequence length
        self.T1 = T0 // 2     # level-1 sequence length
        self.HC = HC          # half-chunk steps (ring buffer half)
        self.mm_bf16 = mm_bf16
        self.C0 = self.T0 * NB
        self.C1 = self.T1 * NB
        self.NCH0 = self.T0 // (2 * HC)   # chunk loop counts
        self.NCH1 = self.T1 // (2 * HC)
        assert self.T0 % (2 * HC) == 0 and self.T1 % (2 * HC) == 0
        self.wdt = BF16 if mm_bf16 else F32


def _dram_in(nc, name, shape, dt=F32):
    return nc.declare_dram_parameter(name, list(shape), dt, isOutput=False)


# ----------------------------------------------------------------------------
# program builder (one SPMD core)
# ----------------------------------------------------------------------------

def build_program(cfg: Cfg):
    nc = bacc.Bacc()
    wdt = cfg.wdt
    C0, C1, HC = cfg.C0, cfg.C1, cfg.HC
    CHC = HC * NB  # columns per half-chunk

    # ---------------- dram I/O ----------------
    xf = _dram_in(nc, "xf", (128, 2, C0))
    xfb = _dram_in(nc, "xfb", (128, 2, C0), wdt)
    eps0f = _dram_in(nc, "eps0f", (128, C0))
    eps1f = _dram_in(nc, "eps1f", (128, C1))
    xsl = _dram_in(nc, "xsl", (1, NB))

    wnames = {
        "w_enc0": (128, 2, 256), "w_enc1": (128, 2, 256),
        "w_gz0": (128, 1, 256), "w_gc0": (128, 3, 256),
        "w_ih0": (128, 2, 768), "w_hh0": (128, 2, 768),
        "w_q1a0": (128, 2, 256), "w_q1b0": (128, 2, 256),
        "w_q2_0": (128, 2, 256), "w_q3_0": (128, 2, 256),
        "w_p1_0": (128, 2, 256), "w_p2_0": (128, 2, 256),
        "w_gz1": (128, 1, 256),
        "w_ih1": (128, 2, 768), "w_hh1": (128, 2, 768),
        "w_q1a1": (128, 2, 256), "w_q1b1": (128, 2, 256),
        "w_q2_1": (128, 2, 256), "w_q3_1": (128, 2, 256),
        "w_p1_1": (128, 2, 256), "w_p2_1": (128, 2, 256),
        "w_dec1": (128, 3, 256), "w_dec2": (128, 2, 512),
    }
    wd = {n: _dram_in(nc, n, s, wdt) for n, s in wnames.items()}

    bcnames = {  # per-partition bias columns [128, nh]
        "bc_enc0": 2, "bc_enc1": 2, "bc_g0": 2, "bc_g1": 2,
        "bc_q1_0": 2, "bc_q1_1": 2,
        "bc_p1_0": 2, "bc_p2_0": 2, "bc_p1_1": 2, "bc_p2_1": 2,
        "bc_dec1": 2, "bc_dec2": 4,
    }
    bcd = {n: _dram_in(nc, n, (128, nh)) for n, nh in bcnames.items()}

    pknames = {  # replicated bias packs [128, nh, NB]
        "pk_rz0": 4, "pk_hn0": 2, "pk_in0": 2, "pk_q2_0": 2, "pk_q3_0": 2,
        "pk_rz1": 4, "pk_hn1": 2, "pk_in1": 2, "pk_q2_1": 2, "pk_q3_1": 2,
    }
    pkd = {n: _dram_in(nc, n, (128, nh, NB)) for n, nh in pknames.items()}

    out_d = nc.declare_dram_parameter("out", [4, NB], F32, isOutput=True)

    # internal HBM scratch
    preq0_d = nc.dram_tensor("preq0_d", [128, 2, C0], F32)
    preq1_d = nc.dram_tensor("preq1_d", [128, 2, C1], F32)
    prec0_d = nc.dram_tensor("prec0_d", [128, 2, C0], F32)
    hs0_d = nc.dram_tensor("hs0_d", [128, 2, C0], wdt)
    zs0_d = nc.dram_tensor("zs0_d", [128, C0], wdt)
    q30_d = nc.dram_tensor("q30_d", [128, 2, C0], F32)
    hs1_d = nc.dram_tensor("hs1_d", [128, 2, C1], wdt)
    zs1_d = nc.dram_tensor("zs1_d", [128, C1], wdt)
    q31_d = nc.dram_tensor("q31_d", [128, 2, C1], F32)

    with TileContext(nc) as tc:
        import contextlib
        stack = contextlib.ExitStack()
        with stack:
            const = stack.enter_context(tc.tile_pool(name="const", bufs=1))

            # resident weights / biases
            wsb = {}
            for n, s in wnames.items():
                wsb[n] = const.tile(list(s), wdt, tag=n, name=n)
                nc.sync.dma_start(out=wsb[n], in_=wd[n][:])
            bcs = {}
            for n, nh in bcnames.items():
                bcs[n] = const.tile([128, nh], F32, tag=n, name=n)
                nc.sync.dma_start(out=bcs[n], in_=bcd[n][:])
            pks = {}
            for n, nh in pknames.items():
                pks[n] = const.tile([128, nh, NB], F32, tag=n, name=n)
                nc.sync.dma_start(out=pks[n], in_=pkd[n][:])
            ones = const.tile([128, 1], F32, tag="ones")
            nc.vector.memset(ones, 1.0)

            # dependency-free ACT warmup: carries the implicit table load
            # (and pins the natural_log_exp set before the scan loops).
            warm = const.tile([1, 8], F32, tag="warm")
            nc.vector.memset(warm, 1.0)
            nc.scalar.activation(warm, warm, AF.Ln)
            nc.scalar.activation(warm, warm, AF.Exp)
            nc.scalar.activation(warm, warm, AF.Relu)

            NCHK0 = C0 // 512   # column chunks for batched phases
            NCHK1 = C1 // 512

            # masking machinery: per-chunk masked accumulation into [1, NB]
            # xr: x_sl broadcast over one 512-col chunk; trow{s}: time value
            # per column (scaled by the level's time factor) for chunk 0.
            xr = const.tile([1, 512], F32, tag="xr")
            x0ap = xsl[:, :]
            nc.sync.dma_start(out=xr, in_=bass.AP(
                tensor=x0ap.tensor, offset=x0ap.offset,
                ap=[x0ap.ap[0], [0, 512 // NB], [1, NB]]))
            trow = {}
            for tf in (1, 2):
                trow[tf] = const.tile([1, 512], F32, tag=f"trow{tf}", name=f"trow{tf}")
                nc.gpsimd.iota(trow[tf], pattern=[[tf, 512 // NB], [0, NB]],
                               base=0, channel_multiplier=0,
                               allow_small_or_imprecise_dtypes=True)
            res_acc = {}
            for rn in ("lp", "kl0", "kl1"):
                res_acc[rn] = const.tile([1, NB], F32, tag=f"res_{rn}", name=f"res_{rn}")
                nc.vector.memset(res_acc[rn], 0.0)

            def masked_accum(pool, csum_ps, chunk_idx, tf, acc):
                """acc += per-sample sums of csum_ps [1,512] masked to t < x_sl.

                column t-values are chunk_idx*(512//NB)*tf + trow[tf]."""
                base = chunk_idx * (512 // NB) * tf
                thr = pool.tile([1, 512], F32, tag="thr")
                nc.vector.tensor_scalar_add(thr, xr, float(-base))
                mk = pool.tile([1, 512], F32, tag="mk")
                nc.vector.tensor_tensor(mk, trow[tf], thr, OP.is_lt)
                nc.vector.tensor_mul(mk, mk, csum_ps)
                mv = mk.rearrange("p (t b) -> p b t", b=NB)
                cs = pool.tile([1, NB], F32, tag="cs")
                nc.vector.tensor_reduce(cs, mv, axis=mybir.AxisListType.X, op=OP.add)
                nc.vector.tensor_add(acc, acc, cs)

            # ---------------- P1/P2: encoders -> preq0/preq1 in HBM --------
            with tc.tile_pool(name="enc", bufs=3) as encp, \
                 tc.tile_pool(name="encps", bufs=4, space="PSUM") as encps:
                for c in range(NCHK0):
                    xc = encp.tile([128, 2, 512], wdt, tag="xc")
                    nc.sync.dma_start(out=xc, in_=xfb[:, :, ts(c, 512)])
                    e0 = encp.tile([128, 2, 512], wdt, tag="e0")
                    for m in range(2):
                        ps = encps.tile([128, 512], F32, tag="ps")
                        for k in range(2):
                            nc.tensor.matmul(ps, lhsT=wsb["w_enc0"][:, k, ts(m, 128)],
                                             rhs=xc[:, k, :], start=(k == 0), stop=(k == 1))
                        nc.scalar.activation(e0[:, m, :], ps, AF.Relu,
                                             bias=bcs["bc_enc0"][:, m:m + 1])
                    pq = encp.tile([128, 2, 512], F32, tag="pq")
                    for m in range(2):
                        ps = encps.tile([128, 512], F32, tag="ps")
                        for k in range(2):
                            nc.tensor.matmul(ps, lhsT=wsb["w_q1b0"][:, k, ts(m, 128)],
                                             rhs=e0[:, k, :], start=(k == 0), stop=(k == 1))
                        nc.scalar.activation(pq[:, m, :], ps, AF.Identity,
                                             bias=bcs["bc_q1_0"][:, m:m + 1])
                    nc.sync.dma_start(out=preq0_d[:, :, ts(c, 512)], in_=pq)

                for c in range(NCHK1):
                    # pooled = x[2k] + x[2k+1]  (the 0.5 is folded into w_enc1)
                    xc = encp.tile([128, 2, 1024], wdt, tag="xc2")
                    nc.sync.dma_start(out=xc, in_=xfb[:, :, ds(c * 1024, 1024)])
                    xv = xc.rearrange("p h (k two b) -> p h k two b", two=2, b=NB)
                    pooled = encp.tile([128, 2, 512], wdt, tag="pooled")
                    pv = pooled.rearrange("p h (k b) -> p h k b", b=NB)
                    nc.vector.tensor_add(pv, xv[:, :, :, 0, :], xv[:, :, :, 1, :])
                    e1 = encp.tile([128, 2, 512], wdt, tag="e1")
                    for m in range(2):
                        ps = encps.tile([128, 512], F32, tag="ps")
                        for k in range(2):
                            nc.tensor.matmul(ps, lhsT=wsb["w_enc1"][:, k, ts(m, 128)],
                                             rhs=pooled[:, k, :], start=(k == 0), stop=(k == 1))
                        nc.scalar.activation(e1[:, m, :], ps, AF.Relu,
                                             bias=bcs["bc_enc1"][:, m:m + 1])
                    pq = encp.tile([128, 2, 512], F32, tag="pq1")
                    for m in range(2):
                        ps = encps.tile([128, 512], F32, tag="ps")
                        for k in range(2):
                            nc.tensor.matmul(ps, lhsT=wsb["w_q1b1"][:, k, ts(m, 128)],
                                             rhs=e1[:, k, :], start=(k == 0), stop=(k == 1))
                        nc.scalar.activation(pq[:, m, :], ps, AF.Identity,
                                             bias=bcs["bc_q1_1"][:, m:m + 1])
                    nc.sync.dma_start(out=preq1_d[:, :, ts(c, 512)], in_=pq)

            # ---------------- P3: level-1 scan ------------------------------
            _scan_level(tc, nc, cfg, lvl=1, wsb=wsb, bcs=bcs, pks=pks,
                        eps_d=eps1f, preq_d=preq1_d, prec_d=None,
                        hs_out=hs1_d, zs_out=zs1_d, q3_out=q31_d,
                        nch=cfg.NCH1)

            # ---------------- P4: prec0 = Wgc0 @ repeat(ctx1) + bg0 ---------
            with tc.tile_pool(name="pc", bufs=3) as pcp, \
                 tc.tile_pool(name="pcps", bufs=4, space="PSUM") as pcps:
                for c in range(NCHK0):
                    # source L1 cols: k in [c*32, c*32+32) repeated 2x -> 512
                    k0 = c * 256  # = (c*512)//2 base column in L1 arrays
                    zrep = pcp.tile([128, 512], wdt, tag="zrep")
                    hrep = pcp.tile([128, 2, 512], wdt, tag="hrep")
                    zsl = zs1_d[:, ds(k0, 256)]
                    zv = zrep.rearrange("p (k two b) -> p k two b", two=2, b=NB)
                    for dup in range(2):
                        nc.sync.dma_start(out=zv[:, :, dup, :], in_=zsl)
                    hsl = hs1_d[:, :, ds(k0, 256)]
                    hv = hrep.rearrange("p h (k two b) -> p h k two b", two=2, b=NB)
                    for hh in range(2):
                        for dup in range(2):
                            nc.sync.dma_start(out=hv[:, hh, :, dup, :], in_=hsl[:, hh, :])
                    pc = pcp.tile([128, 2, 512], F32, tag="pc")
                    for m in range(2):
                        ps = pcps.tile([128, 512], F32, tag="ps")
                        nc.tensor.matmul(ps, lhsT=wsb["w_gc0"][:, 0, ts(m, 128)],
                                         rhs=zrep, start=True, stop=False)
                        nc.tensor.matmul(ps, lhsT=wsb["w_gc0"][:, 1, ts(m, 128)],
                                         rhs=hrep[:, 0, :], start=False, stop=False)
                        nc.tensor.matmul(ps, lhsT=wsb["w_gc0"][:, 2, ts(m, 128)],
                                         rhs=hrep[:, 1, :], start=False, stop=True)
                        nc.scalar.activation(pc[:, m, :], ps, AF.Identity,
                                             bias=bcs["bc_g0"][:, m:m + 1])
                    nc.sync.dma_start(out=prec0_d[:, :, ts(c, 512)], in_=pc)

            # ---------------- P5: level-0 scan ------------------------------
            _scan_level(tc, nc, cfg, lvl=0, wsb=wsb, bcs=bcs, pks=pks,
                        eps_d=eps0f, preq_d=preq0_d, prec_d=prec0_d,
                        hs_out=hs0_d, zs_out=zs0_d, q3_out=q30_d,
                        nch=cfg.NCH0)

            # ---------------- P6: priors + KL rows --------------------------
            for lvl, nchk, hs_d, q3_d, klacc in (
                    (0, NCHK0, hs0_d, q30_d, res_acc["kl0"]),
                    (1, NCHK1, hs1_d, q31_d, res_acc["kl1"])):
                sfx = str(lvl)
                with tc.tile_pool(name=f"kl{lvl}", bufs=1) as klp, \
                     tc.tile_pool(name=f"klps{lvl}", bufs=2, space="PSUM") as klps:
                    for c in range(nchk):
                        hc_ = klp.tile([128, 2, 512], wdt, tag="hc")
                        nc.sync.dma_start(out=hc_, in_=hs_d[:, :, ts(c, 512)])
                        q3c = klp.tile([128, 2, 512], F32, tag="q3c")
                        nc.sync.dma_start(out=q3c, in_=q3_d[:, :, ts(c, 512)])
                        p1t = klp.tile([128, 2, 512], wdt, tag="p1t")
                        for m in range(2):
                            ps = klps.tile([128, 512], F32, tag="ps")
                            for k in range(2):
                                nc.tensor.matmul(ps, lhsT=wsb["w_p1_" + sfx][:, k, ts(m, 128)],
                                                 rhs=hc_[:, k, :], start=(k == 0), stop=(k == 1))
                            nc.scalar.activation(p1t[:, m, :], ps, AF.Relu,
                                                 bias=bcs["bc_p1_" + sfx][:, m:m + 1])
                        # p2: pmu half (m=0), praw half (m=1)
                        p2ps = []
                        for m in range(2):
                            ps = klps.tile([128, 512], F32, tag="p2", bufs=4)
                            for k in range(2):
                                nc.tensor.matmul(ps, lhsT=wsb["w_p2_" + sfx][:, k, ts(m, 128)],
                                                 rhs=p1t[:, k, :], start=(k == 0), stop=(k == 1))
                            p2ps.append(ps)
                        pmu = klp.tile([128, 512], F32, tag="pmu")
                        nc.scalar.activation(pmu, p2ps[0], AF.Identity,
                                             bias=bcs["bc_p2_" + sfx][:, 0:1])
                        # psd = softplus(praw + b) + 1e-4
                        epr = klp.tile([128, 512], F32, tag="epr")
                        nc.scalar.activation(epr, p2ps[1], AF.Exp,
                                             bias=bcs["bc_p2_" + sfx][:, 1:2])
                        nc.vector.tensor_scalar_add(epr, epr, 1.0)
                        spp = klp.tile([128, 512], F32, tag="spp")
                        nc.scalar.activation(spp, epr, AF.Ln)
                        psd = klp.tile([128, 512], F32, tag="psd")
                        nc.vector.tensor_scalar_add(psd, spp, 1e-4)
                        lpsd = klp.tile([128, 512], F32, tag="lpsd")
                        nc.scalar.activation(lpsd, psd, AF.Ln)
                        # qsd = softplus(q3[:,1,:]) + 1e-4
                        eq = klp.tile([128, 512], F32, tag="eq")
                        nc.scalar.activation(eq, q3c[:, 1, :], AF.Exp)
                        nc.vector.tensor_scalar_add(eq, eq, 1.0)
                        spq = klp.tile([128, 512], F32, tag="spq")
                        nc.scalar.activation(spq, eq, AF.Ln)
                        qsd = klp.tile([128, 512], F32, tag="qsd")
                        nc.vector.tensor_scalar_add(qsd, spq, 1e-4)
                        lqsd = klp.tile([128, 512], F32, tag="lqsd")
                        nc.scalar.activation(lqsd, qsd, AF.Ln)
                        # kl = lpsd - lqsd + 0.5*(qsd^2 + (qmu-pmu)^2)/psd^2
                        dl = klp.tile([128, 512], F32, tag="dl")
                        nc.vector.tensor_sub(dl, lpsd, lqsd)
                        rp = klp.tile([128, 512], F32, tag="rp")
                        scr = klp.tile([128, 512], F32, tag="scr")
                        nc.vector.reciprocal_approx_accurate(rp, psd, scr)
                        nc.vector.tensor_mul(rp, rp, rp)
                        q2_ = klp.tile([128, 512], F32, tag="q2_")
                        nc.vector.tensor_mul(q2_, qsd, qsd)
                        dmu = klp.tile([128, 512], F32, tag="dmu")
                        nc.vector.tensor_sub(dmu, q3c[:, 0, :], pmu)
                        nc.vector.tensor_mul(dmu, dmu, dmu)
                        nc.vector.tensor_add(q2_, q2_, dmu)
                        nc.vector.tensor_mul(q2_, q2_, rp)
                        kl = klp.tile([128, 512], F32, tag="kl")
                        nc.vector.scalar_tensor_tensor(kl, q2_, 0.5, dl, OP.mult, OP.add)
                        # column sum over Z=128 partitions
                        ps = klps.tile([1, 512], F32, tag="cs")
                        nc.tensor.matmul(ps, lhsT=ones, rhs=kl, start=True, stop=True)
                        masked_accum(klp, ps, c, 1 if lvl == 0 else 2, klacc)

            # ---------------- P7: decoder + likelihood rows ----------------
            with tc.tile_pool(name="dec", bufs=2) as dcp, \
                 tc.tile_pool(name="decps", bufs=2, space="PSUM") as dcps:
                for c in range(NCHK0):
                    zc = dcp.tile([128, 512], wdt, tag="zc")
                    nc.sync.dma_start(out=zc, in_=zs0_d[:, ts(c, 512)])
                    hcx = dcp.tile([128, 2, 512], wdt, tag="hcx")
                    nc.sync.dma_start(out=hcx, in_=hs0_d[:, :, ts(c, 512)])
                    yc = dcp.tile([128, 2, 512], F32, tag="yc")
                    nc.sync.dma_start(out=yc, in_=xf[:, :, ts(c, 512)])
                    dec = dcp.tile([128, 2, 512], wdt, tag="dect")
                    for m in range(2):
                        ps = dcps.tile([128, 512], F32, tag="ps")
                        nc.tensor.matmul(ps, lhsT=wsb["w_dec1"][:, 0, ts(m, 128)],
                                         rhs=zc, start=True, stop=False)
                        nc.tensor.matmul(ps, lhsT=wsb["w_dec1"][:, 1, ts(m, 128)],
                                         rhs=hcx[:, 0, :], start=False, stop=False)
                        nc.tensor.matmul(ps, lhsT=wsb["w_dec1"][:, 2, ts(m, 128)],
                                         rhs=hcx[:, 1, :], start=False, stop=True)
                        nc.scalar.activation(dec[:, m, :], ps, AF.Relu,
                                             bias=bcs["bc_dec1"][:, m:m + 1])
                    raws = []
                    for m in range(4):
                        ps = dcps.tile([128, 512], F32, tag="raw", bufs=4)
                        for k in range(2):
                            nc.tensor.matmul(ps, lhsT=wsb["w_dec2"][:, k, ds(m * 128, 128)],
                                             rhs=dec[:, k, :], start=(k == 0), stop=(k == 1))
                        raws.append(ps)
                    lps = dcps.tile([1, 512], F32, tag="lsum")
                    for m in range(2):
                        mu = dcp.tile([128, 512], F32, tag="mu")
                        nc.scalar.activation(mu, raws[m], AF.Identity,
                                             bias=bcs["bc_dec2"][:, m:m + 1])
                        esd = dcp.tile([128, 512], F32, tag="esd")
                        nc.scalar.activation(esd, raws[m + 2], AF.Exp,
                                             bias=bcs["bc_dec2"][:, m + 2:m + 3])
                        nc.vector.tensor_scalar_add(esd, esd, 1.0)
                        sd = dcp.tile([128, 512], F32, tag="sd")
                        nc.scalar.activation(sd, esd, AF.Ln)
                        nc.vector.tensor_scalar_add(sd, sd, 1e-4)
                        lsd = dcp.tile([128, 512], F32, tag="lsd")
                        nc.scalar.activation(lsd, sd, AF.Ln)
                        rsd = dcp.tile([128, 512], F32, tag="rsd")
                        scr = dcp.tile([128, 512], F32, tag="scr2")
                        nc.vector.reciprocal_approx_accurate(rsd, sd, scr)
                        dy = dcp.tile([128, 512], F32, tag="dy")
                        nc.vector.tensor_sub(dy, yc[:, m, :], mu)
                        nc.vector.tensor_mul(dy, dy, rsd)
                        nc.vector.tensor_mul(dy, dy, dy)
                        lpel = dcp.tile([128, 512], F32, tag="lpel")
                        nc.vector.scalar_tensor_tensor(lpel, dy, -0.5, lsd,
                                                       OP.mult, OP.subtract)
                        nc.tensor.matmul(lps, lhsT=ones, rhs=lpel,
                                         start=(m == 0), stop=(m == 1))
                    masked_accum(dcp, lps, c, 1, res_acc["lp"])

            # ---------------- P8: write per-sample partial sums ------------
            with tc.tile_pool(name="fin", bufs=1) as fin:
                zrow = fin.tile([1, NB], F32, tag="zrow")
                nc.vector.memset(zrow, 0.0)
                for row, rn in ((0, "lp"), (1, "kl0"), (2, "kl1")):
                    nc.sync.dma_start(out=out_d[row:row + 1, :], in_=res_acc[rn])
                nc.sync.dma_start(out=out_d[3:4, :], in_=zrow)

    return nc.declare_dram_parameter(name, list(shape), dt, isOutput=False)


# ----------------------------------------------------------------------------
# program builder (one SPMD core)
# ----------------------------------------------------------------------------

def build_program(cfg: Cfg):
    nc = bacc.Bacc()
    wdt = cfg.wdt
    C0, C1, HC = cfg.C0, cfg.C1, cfg.HC
    CHC = HC * NB  # columns per half-chunk

    # ---------------- dram I/O ----------------
    xf = _dram_in(nc, "xf", (128, 2, C0))
    xfb = _dram_in(nc, "xfb", (128, 2, C0), wdt)
    eps0f = _dram_in(nc, "eps0f", (128, C0))
    eps1f = _dram_in(nc, "eps1f", (128, C1))
    xsl = _dram_in(nc, "xsl", (1, NB))

    wnames = {
        "w_enc0": (128, 2, 256), "w_enc1": (128, 2, 256),
        "w_gz0": (128, 1, 256), "w_gc0": (128, 3, 256),
        "w_ih0": (128, 2, 768), "w_hh0": (128, 2, 768),
        "w_q1a0": (128, 2, 256), "w_q1b0": (128, 2, 256),
        "w_q2_0": (128, 2, 256), "w_q3_0": (128, 2, 256),
        "w_p1_0": (128, 2, 256), "w_p2_0": (128, 2, 256),
        "w_gz1": (128, 1, 256),
        "w_ih1": (128, 2, 768), "w_hh1": (128, 2, 768),
        "w_q1a1": (128, 2, 256), "w_q1b1": (128, 2, 256),
        "w_q2_1": (128, 2, 256), "w_q3_1": (128, 2, 256),
        "w_p1_1": (128, 2, 256), "w_p2_1": (128, 2, 256),
        "w_dec1": (128, 3, 256), "w_dec2": (128, 2, 512),
    }
    wd = {n: _dram_in(nc, n, s, wdt) for n, s in wnames.items()}

    bcnames = {  # per-partition bias columns [128, nh]
        "bc_enc0": 2, "bc_enc1": 2, "bc_g0": 2, "bc_g1": 2,
        "bc_q1_0": 2, "bc_q1_1": 2,
        "bc_p1_0": 2, "bc_p2_0": 2, "bc_p1_1": 2, "bc_p2_1": 2,
        "bc_dec1": 2, "bc_dec2": 4,
    }
    bcd = {n: _dram_in(nc, n, (128, nh)) for n, nh in bcnames.items()}

    pknames = {  # replicated bias packs [128, nh, NB]
        "pk_rz0": 4, "pk_hn0": 2, "pk_in0": 2, "pk_q2_0": 2, "pk_q3_0": 2,
        "pk_rz1": 4, "pk_hn1": 2, "pk_in1": 2, "pk_q2_1": 2, "pk_q3_1": 2,
    }
    pkd = {n: _dram_in(nc, n, (128, nh, NB)) for n, nh in pknames.items()}

    out_d = nc.declare_dram_parameter("out", [4, NB], F32, isOutput=True)

    # internal HBM scratch
    preq0_d = nc.dram_tensor("preq0_d", [128, 2, C0], F32)
    preq1_d = nc.dram_tensor("preq1_d", [128, 2, C1], F32)
    prec0_d = nc.dram_tensor("prec0_d", [128, 2, C0], F32)
    hs0_d = nc.dram_tensor("hs0_d", [128, 2, C0], wdt)
    zs0_d = nc.dram_tensor("zs0_d", [128, C0], wdt)
    q30_d = nc.dram_tensor("q30_d", [128, 2, C0], F32)
    hs1_d = nc.dram_tensor("hs1_d", [128, 2, C1], wdt)
    zs1_d = nc.dram_tensor("zs1_d", [128, C1], wdt)
    q31_d = nc.dram_tensor("q31_d", [128, 2, C1], F32)

    with TileContext(nc) as tc:
        import contextlib
        stack = contextlib.ExitStack()
        with stack:
            const = stack.enter_context(tc.tile_pool(name="const", bufs=1))

            # resident weights / biases
            wsb = {}
            for n, s in wnames.items():
                wsb[n] = const.tile(list(s), wdt, tag=n, name=n)
                nc.sync.dma_start(out=wsb[n], in_=wd[n][:])
            bcs = {}
            for n, nh in bcnames.items():
                bcs[n] = const.tile([128, nh], F32, tag=n, name=n)
                nc.sync.dma_start(out=bcs[n], in_=bcd[n][:])
            pks = {}
            for n, nh in pknames.items():
                pks[n] = const.tile([128, nh, NB], F32, tag=n, name=n)
                nc.sync.dma_start(out=pks[n], in_=pkd[n][:])
            ones = const.tile([128, 1], F32, tag="ones")
            nc.vector.memset(ones, 1.0)

            # dependency-free ACT warmup: carries the implicit table load
            # (and pins the natural_log_exp set before the scan loops).
            warm = const.tile([1, 8], F32, tag="warm")
            nc.vector.memset(warm, 1.0)
            nc.scalar.activation(warm, warm, AF.Ln)
            nc.scalar.activation(warm, warm, AF.Exp)
            nc.scalar.activation(warm, warm, AF.Relu)

            NCHK0 = C0 // 512   # column chunks for batched phases
            NCHK1 = C1 // 512

            # masking machinery: per-chunk masked accumulation into [1, NB]
            # xr: x_sl broadcast over one 512-col chunk; trow{s}: time value
            # per column (scaled by the level's time factor) for chunk 0.
            xr = const.tile([1, 512], F32, tag="xr")
            x0ap = xsl[:, :]
            nc.sync.dma_start(out=xr, in_=bass.AP(
                tensor=x0ap.tensor, offset=x0ap.offset,
                ap=[x0ap.ap[0], [0, 512 // NB], [1, NB]]))
            trow = {}
            for tf in (1, 2):
                trow[tf] = const.tile([1, 512], F32, tag=f"trow{tf}", name=f"trow{tf}")
                nc.gpsimd.iota(trow[tf], pattern=[[tf, 512 // NB], [0, NB]],
                               base=0, channel_multiplier=0,
                               allow_small_or_imprecise_dtypes=True)
            res_acc = {}
            for rn in ("lp", "kl0", "kl1"):
                res_acc[rn] = const.tile([1, NB], F32, tag=f"res_{rn}", name=f"res_{rn}")
                nc.vector.memset(res_acc[rn], 0.0)

            def masked_accum(pool, csum_ps, chunk_idx, tf, acc):
                """acc += per-sample sums of csum_ps [1,512] masked to t < x_sl.

                column t-values are chunk_idx*(512//NB)*tf + trow[tf]."""
                base = chunk_idx * (512 // NB) * tf
                thr = pool.tile([1, 512], F32, tag="thr")
                nc.vector.tensor_scalar_add(thr, xr, float(-base))
                mk = pool.tile([1, 512], F32, tag="mk")
                nc.vector.tensor_tensor(mk, trow[tf], thr, OP.is_lt)
                nc.vector.tensor_mul(mk, mk, csum_ps)
                mv = mk.rearrange("p (t b) -> p b t", b=NB)
                cs = pool.tile([1, NB], F32, tag="cs")
                nc.vector.tensor_reduce(cs, mv, axis=mybir.AxisListType.X, op=OP.add)
                nc.vector.tensor_add(acc, acc, cs)

            # ---------------- P1/P2: encoders -> preq0/preq1 in HBM --------
            with tc.tile_pool(name="enc", bufs=3) as encp, \
                 tc.tile_pool(name="encps", bufs=4, space="PSUM") as encps:
                for c in range(NCHK0):
                    xc = encp.tile([128, 2, 512], wdt, tag="xc")
                    nc.sync.dma_start(out=xc, in_=xfb[:, :, ts(c, 512)])
                    e0 = encp.tile([128, 2, 512], wdt, tag="e0")
                    for m in range(2):
                        ps = encps.tile([128, 512], F32, tag="ps")
                        for k in range(2):
                            nc.tensor.matmul(ps, lhsT=wsb["w_enc0"][:, k, ts(m, 128)],
                                             rhs=xc[:, k, :], start=(k == 0), stop=(k == 1))
                        nc.scalar.activation(e0[:, m, :], ps, AF.Relu,
                                             bias=bcs["bc_enc0"][:, m:m + 1])
                    pq = encp.tile([128, 2, 512], F32, tag="pq")
                    for m in range(2):
                        ps = encps.tile([128, 512], F32, tag="ps")
                        for k in range(2):
                            nc.tensor.matmul(ps, lhsT=wsb["w_q1b0"][:, k, ts(m, 128)],
                                             rhs=e0[:, k, :], start=(k == 0), stop=(k == 1))
                        nc.scalar.activation(pq[:, m, :], ps, AF.Identity,
                                             bias=bcs["bc_q1_0"][:, m:m + 1])
                    nc.sync.dma_start(out=preq0_d[:, :, ts(c, 512)], in_=pq)

                for c in range(NCHK1):
                    # pooled = x[2k] + x[2k+1]  (the 0.5 is folded into w_enc1)
                    xc = encp.tile([128, 2, 1024], wdt, tag="xc2")
                    nc.sync.dma_start(out=xc, in_=xfb[:, :, ds(c * 1024, 1024)])
                    xv = xc.rearrange("p h (k two b) -> p h k two b", two=2, b=NB)
                    pooled = encp.tile([128, 2, 512], wdt, tag="pooled")
                    pv = pooled.rearrange("p h (k b) -> p h k b", b=NB)
                    nc.vector.tensor_add(pv, xv[:, :, :, 0, :], xv[:, :, :, 1, :])
                    e1 = encp.tile([128, 2, 512], wdt, tag="e1")
                    for m in range(2):
                        ps = encps.tile([128, 512], F32, tag="ps")
                        for k in range(2):
                            nc.tensor.matmul(ps, lhsT=wsb["w_enc1"][:, k, ts(m, 128)],
                                             rhs=pooled[:, k, :], start=(k == 0), stop=(k == 1))
                        nc.scalar.activation(e1[:, m, :], ps, AF.Relu,
                                             bias=bcs["bc_enc1"][:, m:m + 1])
                    pq = encp.tile([128, 2, 512], F32, tag="pq1")
                    for m in range(2):
                        ps = encps.tile([128, 512], F32, tag="ps")
                        for k in range(2):
                            nc.tensor.matmul(ps, lhsT=wsb["w_q1b1"][:, k, ts(m, 128)],
                                             rhs=e1[:, k, :], start=(k == 0), stop=(k == 1))
                        nc.scalar.activation(pq[:, m, :], ps, AF.Identity,
                                             bias=bcs["bc_q1_1"][:, m:m + 1])
                    nc.sync.dma_start(out=preq1_d[:, :, ts(c, 512)], in_=pq)

            # ---------------- P3: level-1 scan ------------------------------
            _scan_level(tc, nc, cfg, lvl=1, wsb=wsb, bcs=bcs, pks=pks,
                        eps_d=eps1f, preq_d=preq1_d, prec_d=None,
                        hs_out=hs1_d, zs_out=zs1_d, q3_out=q31_d,
                        nch=cfg.NCH1)

            # ---------------- P4: prec0 = Wgc0 @ repeat(ctx1) + bg0 ---------
            with tc.tile_pool(name="pc", bufs=3) as pcp, \
                 tc.tile_pool(name="pcps", bufs=4, space="PSUM") as pcps:
                for c in range(NCHK0):
                    # source L1 cols: k in [c*32, c*32+32) repeated 2x -> 512
                    k0 = c * 256  # = (c*512)//2 base column in L1 arrays
                    zrep = pcp.tile([128, 512], wdt, tag="zrep")
                    hrep = pcp.tile([128, 2, 512], wdt, tag="hrep")
                    zsl = zs1_d[:, ds(k0, 256)]
                    zv = zrep.rearrange("p (k two b) -> p k two b", two=2, b=NB)
                    for dup in range(2):
                        nc.sync.dma_start(out=zv[:, :, dup, :], in_=zsl)
                    hsl = hs1_d[:, :, ds(k0, 256)]
                    hv = hrep.rearrange("p h (k two b) -> p h k two b", two=2, b=NB)
                    for hh in range(2):
                        for dup in range(2):
                            nc.sync.dma_start(out=hv[:, hh, :, dup, :], in_=hsl[:, hh, :])
                    pc = pcp.tile([128, 2, 512], F32, tag="pc")
                    for m in range(2):
                        ps = pcps.tile([128, 512], F32, tag="ps")
                        nc.tensor.matmul(ps, lhsT=wsb["w_gc0"][:, 0, ts(m, 128)],
                                         rhs=zrep, start=True, stop=False)
                        nc.tensor.matmul(ps, lhsT=wsb["w_gc0"][:, 1, ts(m, 128)],
                                         rhs=hrep[:, 0, :], start=False, stop=False)
                        nc.tensor.matmul(ps, lhsT=wsb["w_gc0"][:, 2, ts(m, 128)],
                                         rhs=hrep[:, 1, :], start=False, stop=True)
                        nc.scalar.activation(pc[:, m, :], ps, AF.Identity,
                                             bias=bcs["bc_g0"][:, m:m + 1])
                    nc.sync.dma_start(out=prec0_d[:, :, ts(c, 512)], in_=pc)

            # ---------------- P5: level-0 scan ------------------------------
            _scan_level(tc, nc, cfg, lvl=0, wsb=wsb, bcs=bcs, pks=pks,
                        eps_d=eps0f, preq_d=preq0_d, prec_d=prec0_d,
                        hs_out=hs0_d, zs_out=zs0_d, q3_out=q30_d,
                        nch=cfg.NCH0)

            # ---------------- P6: priors + KL rows --------------------------
            for lvl, nchk, hs_d, q3_d, klacc in (
                    (0, NCHK0, hs0_d, q30_d, res_acc["kl0"]),
                    (1, NCHK1, hs1_d, q31_d, res_acc["kl1"])):
                sfx = str(lvl)
                with tc.tile_pool(name=f"kl{lvl}", bufs=1) as klp, \
                     tc.tile_pool(name=f"klps{lvl}", bufs=2, space="PSUM") as klps:
                    for c in range(nchk):
                        hc_ = klp.tile([128, 2, 512], wdt, tag="hc")
                        nc.sync.dma_start(out=hc_, in_=hs_d[:, :, ts(c, 512)])
                        q3c = klp.tile([128, 2, 512], F32, tag="q3c")
                        nc.sync.dma_start(out=q3c, in_=q3_d[:, :, ts(c, 512)])
                        p1t = klp.tile([128, 2, 512], wdt, tag="p1t")
                        for m in range(2):
                            ps = klps.tile([128, 512], F32, tag="ps")
                            for k in range(2):
                                nc.tensor.matmul(ps, lhsT=wsb["w_p1_" + sfx][:, k, ts(m, 128)],
                                                 rhs=hc_[:, k, :], start=(k == 0), stop=(k == 1))
                            nc.scalar.activation(p1t[:, m, :], ps, AF.Relu,
                                                 bias=bcs["bc_p1_" + sfx][:, m:m + 1])
                        # p2: pmu half (m=0), praw half (m=1)
                        p2ps = []
                        for m in range(2):
                            ps = klps.tile([128, 512], F32, tag="p2", bufs=4)
                            for k in range(2):
                                nc.tensor.matmul(ps, lhsT=wsb["w_p2_" + sfx][:, k, ts(m, 128)],
                                                 rhs=p1t[:, k, :], start=(k == 0), stop=(k == 1))
                            p2ps.append(ps)
                        pmu = klp.tile([128, 512], F32, tag="pmu")
                        nc.scalar.activation(pmu, p2ps[0], AF.Identity,
                                             bias=bcs["bc_p2_" + sfx][:, 0:1])
                        # psd = softplus(praw + b) + 1e-4
                        epr = klp.tile([128, 512], F32, tag="epr")
                        nc.scalar.activation(epr, p2ps[1], AF.Exp,
                                             bias=bcs["bc_p2_" + sfx][:, 1:2])
                        nc.vector.tensor_scalar_add(epr, epr, 1.0)
                        spp = klp.tile([128, 512], F32, tag="spp")
                        nc.scalar.activation(spp, epr, AF.Ln)
                        psd = klp.tile([128, 512], F32, tag="psd")
                        nc.vector.tensor_scalar_add(psd, spp, 1e-4)
                        lpsd = klp.tile([128, 512], F32, tag="lpsd")
                        nc.scalar.activation(lpsd, psd, AF.Ln)
                        # qsd = softplus(q3[:,1,:]) + 1e-4
                        eq = klp.tile([128, 512], F32, tag="eq")
                        nc.scalar.activation(eq, q3c[:, 1, :], AF.Exp)
                        nc.vector.tensor_scalar_add(eq, eq, 1.0)
                        spq = klp.tile([128, 512], F32, tag="spq")
                        nc.scalar.activation(spq, eq, AF.Ln)
                        qsd = klp.tile([128, 512], F32, tag="qsd")
                        nc.vector.tensor_scalar_add(qsd, spq, 1e-4)
                        lqsd = klp.tile([128, 512], F32, tag="lqsd")
                        nc.scalar.activation(lqsd, qsd, AF.Ln)
                        # kl = lpsd - lqsd + 0.5*(qsd^2 + (qmu-pmu)^2)/psd^2
                        dl = klp.tile([128, 512], F32, tag="dl")
                        nc.vector.tensor_sub(dl, lpsd, lqsd)
                        rp = klp.tile([128, 512], F32, tag="rp")
                        scr = klp.tile([128, 512], F32, tag="scr")
                        nc.vector.reciprocal_approx_accurate(rp, psd, scr)
                        nc.vector.tensor_mul(rp, rp, rp)
                        q2_ = klp.tile([128, 512], F32, tag="q2_")
                        nc.vector.tensor_mul(q2_, qsd, qsd)
                        dmu = klp.tile([128, 512], F32, tag="dmu")
                        nc.vector.tensor_sub(dmu, q3c[:, 0, :], pmu)
                        nc.vector.tensor_mul(dmu, dmu, dmu)
                        nc.vector.tensor_add(q2_, q2_, dmu)
                        nc.vector.tensor_mul(q2_, q2_, rp)
                        kl = klp.tile([128, 512], F32, tag="kl")
                        nc.vector.scalar_tensor_tensor(kl, q2_, 0.5, dl, OP.mult, OP.add)
                        # column sum over Z=128 partitions
                        ps = klps.tile([1, 512], F32, tag="cs")
                        nc.tensor.matmul(ps, lhsT=ones, rhs=kl, start=True, stop=True)
                        masked_accum(klp, ps, c, 1 if lvl == 0 else 2, klacc)

            # ---------------- P7: decoder + likelihood rows ----------------
            with tc.tile_pool(name="dec", bufs=2) as dcp, \
                 tc.tile_pool(name="decps", bufs=2, space="PSUM") as dcps:
                for c in range(NCHK0):
                    zc = dcp.tile([128, 512], wdt, tag="zc")
                    nc.sync.dma_start(out=zc, in_=zs0_d[:, ts(c, 512)])
                    hcx = dcp.tile([128, 2, 512], wdt, tag="hcx")
                    nc.sync.dma_start(out=hcx, in_=hs0_d[:, :, ts(c, 512)])
                    yc = dcp.tile([128, 2, 512], F32, tag="yc")
                    nc.sync.dma_start(out=yc, in_=xf[:, :, ts(c, 512)])
                    dec = dcp.tile([128, 2, 512], wdt, tag="dect")
                    for m in range(2):
                        ps = dcps.tile([128, 512], F32, tag="ps")
                        nc.tensor.matmul(ps, lhsT=wsb["w_dec1"][:, 0, ts(m, 128)],
                                         rhs=zc, start=True, stop=False)
                        nc.tensor.matmul(ps, lhsT=wsb["w_dec1"][:, 1, ts(m, 128)],
                                         rhs=hcx[:, 0, :], start=False, stop=False)
                        nc.tensor.matmul(ps, lhsT=wsb["w_dec1"][:, 2, ts(m, 128)],
                                         rhs=hcx[:, 1, :], start=False, stop=True)
                        nc.scalar.activation(dec[:, m, :], ps, AF.Relu,
                                             bias=bcs["bc_dec1"][:, m:m + 1])
                    raws = []
                    for m in range(4):
                        ps = dcps.tile([128, 512], F32, tag="raw", bufs=4)
                        for k in range(2):
                            nc.tensor.matmul(ps, lhsT=wsb["w_dec2"][:, k, ds(m * 128, 128)],
                                             rhs=dec[:, k, :], start=(k == 0), stop=(k == 1))
                        raws.append(ps)
                    lps = dcps.tile([1, 512], F32, tag="lsum")
                    for m in range(2):
                        mu = dcp.tile([128, 512], F32, tag="mu")
                        nc.scalar.activation(mu, raws[m], AF.Identity,
                                             bias=bcs["bc_dec2"][:, m:m + 1])
                        esd = dcp.tile([128, 512], F32, tag="esd")
                        nc.scalar.activation(esd, raws[m + 2], AF.Exp,
                                             bias=bcs["bc_dec2"][:, m + 2:m + 3])
                        nc.vector.tensor_scalar_add(esd, esd, 1.0)
                        sd = dcp.tile([128, 512], F32, tag="sd")
                        nc.scalar.activation(sd, esd, AF.Ln)
                        nc.vector.tensor_scalar_add(sd, sd, 1e-4)
                        lsd = dcp.tile([128, 512], F32, tag="lsd")
                        nc.scalar.activation(lsd, sd, AF.Ln)
                        rsd = dcp.tile([128, 512], F32, tag="rsd")
                        scr = dcp.tile([128, 512], F32, tag="scr2")
                        nc.vector.reciprocal_approx_accurate(rsd, sd, scr)
                        dy = dcp.tile([128, 512], F32, tag="dy")
                        nc.vector.tensor_sub(dy, yc[:, m, :], mu)
                        nc.vector.tensor_mul(dy, dy, rsd)
                        nc.vector.tensor_mul(dy, dy, dy)
                        lpel = dcp.tile([128, 512], F32, tag="lpel")
                        nc.vector.scalar_tensor_tensor(lpel, dy, -0.5, lsd,
                                                       OP.mult, OP.subtract)
                        nc.tensor.matmul(lps, lhsT=ones, rhs=lpel,
                                         start=(m == 0), stop=(m == 1))
                    masked_accum(dcp, lps, c, 1, res_acc["lp"])

            # ---------------- P8: masks, per-sample reduce, output ---------
            with tc.tile_pool(name="fin", bufs=1) as fin:
                ti0 = fin.tile([1, C0], I32, tag="ti0")
                nc.gpsimd.iota(ti0, pattern=[[1, cfg.T0], [0, NB]],
                               base=0, channel_multiplier=0)
                tf0 = fin.tile([1, C0], F32, tag="tf0")
                nc.vector.tensor_copy(tf0, ti0)
                ti1 = fin.tile([1, C1], I32, tag="ti1")
                nc.gpsimd.iota(ti1, pattern=[[2, cfg.T1], [0, NB]],
                               base=0, channel_multiplier=0)
                tf1 = fin.tile([1, C1], F32, tag="tf1")
                nc.vector.tensor_copy(tf1, ti1)
                xr0 = fin.tile([1, C0], F32, tag="xr0")
                x0 = xsl[:, :]
                nc.sync.dma_start(out=xr0, in_=bass.AP(
                    tensor=x0.tensor, offset=x0.offset,
                    ap=[x0.ap[0], [0, cfg.T0], [1, NB]]))
                xr1 = fin.tile([1, C1], F32, tag="xr1")
                nc.sync.dma_start(out=xr1, in_=bass.AP(
                    tensor=x0.tensor, offset=x0.offset,
                    ap=[x0.ap[0], [0, cfg.T1], [1, NB]]))
                m0 = fin.tile([1, C0], F32, tag="m0")
                nc.vector.tensor_tensor(m0, tf0, xr0, OP.is_lt)
                m1 = fin.tile([1, C1], F32, tag="m1")
                nc.vector.tensor_tensor(m1, tf1, xr1, OP.is_lt)

                nc.vector.tensor_mul(lprow, lprow, m0)
                nc.vector.tensor_mul(kl0row, kl0row, m0)
                nc.vector.tensor_mul(kl1row, kl1row, m1)

                res = fin.tile([4, NB], F32, tag="res")
                nc.vector.memset(res, 0.0)
                for row, src, tt in ((0, lprow, cfg.T0), (1, kl0row, cfg.T0),
                                     (2, kl1row, cfg.T1)):
                    v = src.rearrange("p (t b) -> p b t", b=NB)
                    nc.vector.tensor_reduce(res[row:row + 1, :], v,
                                            axis=mybir.AxisListType.X, op=OP.add)
                nc.sync.dma_start(out=out_d[:, :], in_=res)

    nc.compile()
    return nc


# ----------------------------------------------------------------------------
# the sequential GRU scan for one level
# ----------------------------------------------------------------------------

def _scan_level(tc, nc, cfg, lvl, wsb, bcs, pks, eps_d, preq_d, prec_d,
                hs_out, zs_out, q3_out, nch):
    HC = cfg.HC
    CHC = HC * NB          # columns per half-chunk
    CB = 2 * CHC           # columns per body (full chunk)
    sfx = str(lvl)
    wdt = cfg.wdt
    w_gz = wsb["w_gz" + sfx]
    w_ih = wsb["w_ih" + sfx]
    w_hh = wsb["w_hh" + sfx]
    w_q1a = wsb["w_q1a" + sfx]
    w_q2 = wsb["w_q2_" + sfx]
    w_q3 = wsb["w_q3_" + sfx]
    pk_rz = pks["pk_rz" + sfx]
    pk_hn = pks["pk_hn" + sfx]
    pk_in = pks["pk_in" + sfx]
    pk_q2 = pks["pk_q2_" + sfx]
    pk_q3 = pks["pk_q3_" + sfx]

    import contextlib
    with contextlib.ExitStack() as st:
        rings = st.enter_context(tc.tile_pool(name=f"ring{lvl}", bufs=1))
        # combined GRU bias pack matching PABC psum layout:
        # [rz gates (4) | in=bih_n (2) | hn_g=bhh_n (2)]
        pk_all = rings.tile([128, 8, NB], F32, tag=f"pkall{lvl}", name=f"pkall{lvl}")
        nc.vector.tensor_copy(pk_all[:, 0:4, :], pk_rz)
        nc.vector.tensor_copy(pk_all[:, 4:6, :], pk_in)
        nc.vector.tensor_copy(pk_all[:, 6:8, :], pk_hn)
        inp = st.enter_context(tc.tile_pool(name=f"inp{lvl}", bufs=2))
        work = st.enter_context(tc.tile_pool(name=f"wk{lvl}", bufs=4))
        psum = st.enter_context(tc.tile_pool(name=f"ps{lvl}", bufs=1, space="PSUM"))

        # rings: two halves (A/B); fp32 state + (optional) bf16 matmul shadows
        hsr = [rings.tile([128, 2, CHC], F32, tag=f"hs{i}", name=f"hs{i}") for i in range(2)]
        zsr = [rings.tile([128, CHC], F32, tag=f"zs{i}", name=f"zs{i}") for i in range(2)]
        q3r = [rings.tile([128, 2, CHC], F32, tag=f"q3{i}", name=f"q3{i}") for i in range(2)]
        if cfg.mm_bf16:
            hbr = [rings.tile([128, 2, CHC], BF16, tag=f"hb{i}", name=f"hb{i}") for i in range(2)]
            zbr = [rings.tile([128, CHC], BF16, tag=f"zb{i}", name=f"zb{i}") for i in range(2)]
        else:
            hbr, zbr = hsr, zsr
        # zero-init the "previous step" slices (last column group of half B)
        nc.vector.memset(hsr[1][:, :, ds(CHC - NB, NB)], 0.0)
        nc.vector.memset(zsr[1][:, ds(CHC - NB, NB)], 0.0)
        if cfg.mm_bf16:
            nc.vector.memset(hbr[1][:, :, ds(CHC - NB, NB)], 0.0)
            nc.vector.memset(zbr[1][:, ds(CHC - NB, NB)], 0.0)

        with tc.For_i(0, nch, 1, hint_engines=(mybir.EngineType.PE,
                                               mybir.EngineType.DVE,
                                               mybir.EngineType.Activation)) as ci:
            base = ci * CB
            # stream this chunk's per-step inputs
            epst = inp.tile([128, CB], F32, tag="eps")
            nc.sync.dma_start(out=epst, in_=eps_d[:, ds(base, CB)])
            preqt = inp.tile([128, 2, CB], F32, tag="preq")
            nc.sync.dma_start(out=preqt, in_=preq_d[:, :, ds(base, CB)])
            if prec_d is not None:
                prect = inp.tile([128, 2, CB], F32, tag="prec")
                nc.sync.dma_start(out=prect, in_=prec_d[:, :, ds(base, CB)])

            for half in range(2):
                hs_, zs_, q3_ = hsr[half], zsr[half], q3r[half]
                hb_, zb_ = hbr[half], zbr[half]
                hp_, zp_ = hbr[1 - half], zbr[1 - half]
                hpf_ = hsr[1 - half]
                for s in range(HC):
                    co = half * CHC + s * NB          # column offset in chunk
                    so = s * NB                        # offset within ring
                    po = (s - 1) * NB if s > 0 else (HC - 1) * NB
                    h_prev_b = (hb_ if s > 0 else hp_)[:, :, ds(po, NB)]
                    z_prev_b = (zb_ if s > 0 else zp_)[:, ds(po, NB)]
                    h_prev_f = (hs_ if s > 0 else hpf_)[:, :, ds(po, NB)]

                    # ---- PE: gate matmuls ----
                    PABC = psum.tile([128, 8, NB], F32, tag="PABC", bufs=2)
                    Pgz = psum.tile([128, 2, NB], F32, tag="Pgz")
                    for m in range(4):
                        for k in range(2):
                            nc.tensor.matmul(PABC[:, m, :], lhsT=w_hh[:, k, ts(m, 128)],
                                             rhs=h_prev_b[:, k, :],
                                             start=(k == 0), stop=False,
                                             skip_group_check=True)
                    for m in range(2):
                        for k in range(2):
                            nc.tensor.matmul(PABC[:, 6 + m, :],
                                             lhsT=w_hh[:, k, ds((4 + m) * 128, 128)],
                                             rhs=h_prev_b[:, k, :],
                                             start=(k == 0), stop=(k == 1),
                                             skip_group_check=True)
                    for m in range(2):
                        nc.tensor.matmul(Pgz[:, m, :], lhsT=w_gz[:, 0, ts(m, 128)],
                                         rhs=z_prev_b, start=True, stop=True)

                    # ---- gi = relu(gz + prec) ----
                    gi = work.tile([128, 2, NB], wdt, tag="gi")
                    if prec_d is not None:
                        gt = work.tile([128, 2, NB], F32, tag="gt")
                        nc.vector.tensor_add(gt, Pgz, prect[:, :, ds(co, NB)])
                        nc.vector.tensor_scalar_max(gi, gt, 0.0)
                    else:
                        for m in range(2):
                            nc.scalar.activation(gi[:, m, :], Pgz[:, m, :], AF.Relu,
                                                 bias=bcs["bc_g1"][:, m:m + 1])

                    for m in range(4):
                        for k in range(2):
                            nc.tensor.matmul(PABC[:, m, :], lhsT=w_ih[:, k, ts(m, 128)],
                                             rhs=gi[:, k, :],
                                             start=False, stop=(k == 1),
                                             skip_group_check=True)
                    for m in range(2):
                        for k in range(2):
                            nc.tensor.matmul(PABC[:, 4 + m, :],
                                             lhsT=w_ih[:, k, ds((4 + m) * 128, 128)],
                                             rhs=gi[:, k, :],
                                             start=(k == 0), stop=(k == 1),
                                             skip_group_check=True)

                    # ---- fused bias add, then gates and n-path from SBUF ----
                    u = work.tile([128, 8, NB], F32, tag="u")
                    nc.vector.tensor_add(u, PABC, pk_all)
                    eu = work.tile([128, 4, NB], F32, tag="eu")
                    nc.scalar.activation(eu, u[:, 0:4, :], AF.Exp, scale=-1.0)
                    nc.vector.tensor_scalar_add(eu, eu, 1.0)
                    rz = work.tile([128, 4, NB], F32, tag="rz")
                    nc.vector.reciprocal_approx_fast(rz, eu)

                    # n = tanh((in+bin) + r*(hn_g+bhn))
                    a = work.tile([128, 2, NB], F32, tag="a")
                    nc.vector.tensor_mul(a, rz[:, 0:2, :], u[:, 6:8, :])
                    nc.vector.tensor_add(a, a, u[:, 4:6, :])
                    en = work.tile([128, 2, NB], F32, tag="en")
                    nc.scalar.activation(en, a, AF.Exp, scale=2.0)
                    nc.vector.tensor_scalar_add(en, en, 1.0)
                    vn = work.tile([128, 2, NB], F32, tag="vn")
                    nc.vector.reciprocal_approx_fast(vn, en)
                    n = work.tile([128, 2, NB], F32, tag="n")
                    nc.vector.tensor_scalar(n, vn, -2.0, 1.0, OP.mult, OP.add)

                    # ---- h = n + zg*(h_prev - n) ----
                    dtl = work.tile([128, 2, NB], F32, tag="dtl")
                    nc.vector.tensor_sub(dtl, h_prev_f, n)
                    nc.vector.tensor_mul(dtl, rz[:, 2:4, :], dtl)
                    hn = hs_[:, :, ds(so, NB)]
                    nc.vector.tensor_add(hn, n, dtl)
                    if cfg.mm_bf16:
                        nc.vector.tensor_copy(hb_[:, :, ds(so, NB)], hn)
                    hme = hb_[:, :, ds(so, NB)]

                    # ---- q path ----
                    Pq1 = psum.tile([128, 2, NB], F32, tag="Pq1")
                    for m in range(2):
                        for k in range(2):
                            nc.tensor.matmul(Pq1[:, m, :], lhsT=w_q1a[:, k, ts(m, 128)],
                                             rhs=hme[:, k, :], start=(k == 0), stop=(k == 1))
                    q1t = work.tile([128, 2, NB], F32, tag="q1t")
                    nc.vector.tensor_add(q1t, Pq1, preqt[:, :, ds(co, NB)])
                    q1 = work.tile([128, 2, NB], wdt, tag="q1")
                    nc.vector.tensor_scalar_max(q1, q1t, 0.0)

                    Pq2 = psum.tile([128, 2, NB], F32, tag="Pq2")
                    for m in range(2):
                        for k in range(2):
                            nc.tensor.matmul(Pq2[:, m, :], lhsT=w_q2[:, k, ts(m, 128)],
                                             rhs=q1[:, k, :], start=(k == 0), stop=(k == 1))
                    q2t = work.tile([128, 2, NB], F32, tag="q2t")
                    nc.vector.tensor_add(q2t, Pq2, pk_q2)
                    q2 = work.tile([128, 2, NB], wdt, tag="q2")
                    nc.vector.tensor_scalar_max(q2, q2t, 0.0)

                    Pq3 = psum.tile([128, 2, NB], F32, tag="Pq3")
                    for m in range(2):
                        for k in range(2):
                            nc.tensor.matmul(Pq3[:, m, :], lhsT=w_q3[:, k, ts(m, 128)],
                                             rhs=q2[:, k, :], start=(k == 0), stop=(k == 1))
                    q3n = q3_[:, :, ds(so, NB)]
                    nc.vector.tensor_add(q3n, Pq3, pk_q3)

                    # ---- z = qmu + (softplus(qraw)+1e-4)*eps ----
                    es = work.tile([128, NB], F32, tag="es")
                    nc.scalar.activation(es, q3n[:, 1, :], AF.Exp)
                    nc.vector.tensor_scalar_add(es, es, 1.0)
                    sp = work.tile([128, NB], F32, tag="sp")
                    nc.scalar.activation(sp, es, AF.Ln)
                    nc.vector.scalar_tensor_tensor(sp, sp, 1e-4, epst[:, ds(co, NB)],
                                                   OP.add, OP.mult)
                    zn = zs_[:, ds(so, NB)]
                    nc.vector.tensor_add(zn, q3n[:, 0, :], sp)
                    if cfg.mm_bf16:
                        nc.vector.tensor_copy(zb_[:, ds(so, NB)], zn)

                # ---- spill this half-chunk ----
                hofs = base + half * CHC
                nc.sync.dma_start(out=hs_out[:, :, ds(hofs, CHC)],
                                  in_=hb_ if cfg.mm_bf16 else hs_)
                nc.sync.dma_start(out=zs_out[:, ds(hofs, CHC)],
                                  in_=zb_ if cfg.mm_bf16 else zs_)
                nc.sync.dma_start(out=q3_out[:, :, ds(hofs, CHC)], in_=q3_)


# ----------------------------------------------------------------------------
# host side
# ----------------------------------------------------------------------------

def _fmajor(a, nh):
    """[B_, T_, F] -> [128, nh, T_*NB] time-major batch-inner (per core slice)"""
    b_, t_, f_ = a.shape
    out = a.transpose(2, 1, 0).reshape(nh, 128, t_ * b_)
    return np.ascontiguousarray(out.transpose(1, 0, 2))


def _wprep(w, dt):
    k, m_ = w.shape
    out = np.ascontiguousarray(w.reshape(k // 128, 128, m_).transpose(1, 0, 2))
    return out.astype(dt)


def _bcols(b):
    return np.ascontiguousarray(b.reshape(-1, 128).T.astype(np.float32))


def _bpack(b):
    return np.ascontiguousarray(
        np.repeat(_bcols(b)[:, :, None], NB, axis=2).astype(np.float32))


def prep_core_inputs(x, x_sl, eps0, eps1, params, cfg, core):
    import ml_dtypes
    wnp = ml_dtypes.bfloat16 if cfg.mm_bf16 else np.float32
    sl = slice(core * NB, (core + 1) * NB)
    d = {}
    d["xf"] = _fmajor(np.asarray(x[sl], np.float32), 2)
    d["xfb"] = d["xf"].astype(wnp)
    d["eps0f"] = _fmajor(np.asarray(eps0[sl], np.float32), 1).reshape(128, -1)
    d["eps1f"] = _fmajor(np.asarray(eps1[sl], np.float32), 1).reshape(128, -1)
    d["xsl"] = np.asarray(x_sl[sl], np.float32).reshape(1, NB)

    p = params
    c0, c1 = p["cells"][0], p["cells"][1]
    d["w_enc0"] = _wprep(np.asarray(p["enc_W0"], np.float32), wnp)
    d["w_enc1"] = _wprep(0.5 * np.asarray(p["enc_W1"], np.float32), wnp)
    d["bc_enc0"] = _bcols(np.asarray(p["enc_b0"], np.float32))
    d["bc_enc1"] = _bcols(np.asarray(p["enc_b1"], np.float32))
    for i, c in ((0, c0), (1, c1)):
        Wg = np.asarray(c["Wg"], np.float32)
        d[f"w_gz{i}"] = _wprep(Wg[:Z], wnp)
        if i == 0:
            d["w_gc0"] = _wprep(Wg[Z:], wnp)
            d["bc_g0"] = _bcols(np.asarray(c["bg"], np.float32))
        else:
            d["bc_g1"] = _bcols(np.asarray(c["bg"], np.float32))
        d[f"w_ih{i}"] = _wprep(np.asarray(c["Wih"], np.float32), wnp)
        d[f"w_hh{i}"] = _wprep(np.asarray(c["Whh"], np.float32), wnp)
        Wq1 = np.asarray(c["Wq1"], np.float32)
        d[f"w_q1a{i}"] = _wprep(Wq1[:H], wnp)
        d[f"w_q1b{i}"] = _wprep(Wq1[H:], wnp)
        d[f"bc_q1_{i}"] = _bcols(np.asarray(c["bq1"], np.float32))
        d[f"w_q2_{i}"] = _wprep(np.asarray(c["Wq2"], np.float32), wnp)
        d[f"w_q3_{i}"] = _wprep(np.asarray(c["Wq3"], np.float32), wnp)
        d[f"w_p1_{i}"] = _wprep(np.asarray(c["Wp1"], np.float32), wnp)
        d[f"w_p2_{i}"] = _wprep(np.asarray(c["Wp2"], np.float32), wnp)
        d[f"bc_p1_{i}"] = _bcols(np.asarray(c["bp1"], np.float32))
        d[f"bc_p2_{i}"] = _bcols(np.asarray(c["bp2"], np.float32))
        bih = np.asarray(c["bih"], np.float32)
        bhh = np.asarray(c["bhh"], np.float32)
        d[f"pk_rz{i}"] = _bpack((bih + bhh)[:2 * H])
        d[f"pk_hn{i}"] = _bpack(bhh[2 * H:])
        d[f"pk_in{i}"] = _bpack(bih[2 * H:])
        d[f"pk_q2_{i}"] = _bpack(np.asarray(c["bq2"], np.float32))
        d[f"pk_q3_{i}"] = _bpack(np.asarray(c["bq3"], np.float32))
    d["w_dec1"] = _wprep(np.asarray(p["dec_W1"], np.float32), wnp)
    d["bc_dec1"] = _bcols(np.asarray(p["dec_b1"], np.float32))
    d["w_dec2"] = _wprep(np.asarray(p["dec_W2"], np.float32), wnp)
    d["bc_dec2"] = _bcols(np.asarray(p["dec_b2"], np.float32))
    return d


def combine_partials(outs, x_sl, cfg):
    """outs: list of [4, NB] per core -> scalar loss (host reduction)."""
    x_sl = np.asarray(x_sl).astype(np.float64)
    total = 0.0
    for core, o in enumerate(outs):
        sl = slice(core * NB, (core + 1) * NB)
        xs = x_sl[sl]
        lp = o[0].astype(np.float64) - 0.5 * LOG2PI * D * xs
        kl0 = o[1].astype(np.float64) - 0.5 * Z * xs
        kl1 = o[2].astype(np.float64) - 0.5 * Z * np.ceil(xs / 2.0)
        total += np.sum(lp - kl0 - kl1)
    return np.float32(-total / x_sl.sum())


_PROGRAM_CACHE = {}


def _get_program(cfg):
    key = (cfg.T0, cfg.HC, cfg.mm_bf16)
    if key not in _PROGRAM_CACHE:
        _PROGRAM_CACHE[key] = build_program(cfg)
    return _PROGRAM_CACHE[key]


def run(x, x_sl, eps0, eps1, params, cfg=None, trace=False):
    from concourse.bass_utils import run_bass_kernel_spmd
    if cfg is None:
        cfg = Cfg()
    nc = _get_program(cfg)
    in_maps = [prep_core_inputs(x, x_sl, eps0, eps1, params, cfg, c)
               for c in range(NCORES)]
    res = run_bass_kernel_spmd(nc, in_maps, list(range(NCORES)), trace=trace)
    outs = [r["out"] for r in res.results]
    return combine_partials(outs, x_sl, cfg), res


def kernel(x, x_sl, eps0, eps1, params):
    loss, _ = run(x, x_sl, eps0, eps1, params)
    return loss
